# revision 39
# baseline (speedup 1.0000x reference)
"""EvolveGCN (2x GCNConv+GRU + linear head) on 8 Trainium2 NeuronCores.

Strategy: dst-sharded graph parallel, gather-based message routing.
Each core owns 12500 destination nodes mapped onto 12544 slots by a
degree-balancing permutation (equalizes edge count per (group, chunk)
bucket across cores, minimizing SPMD padding). Dense per-node compute is
feature-major. The GCN propagate runs as PE matmuls over 128-edge
windows: psum[128f, W] += M_w.T @ S_w where M_w is a dma_gather'ed
message tile and S_w a host-built selection matrix.

Key perf structure (the kernel is Q7 descriptor-generation bound):
- 4 SWDGE queues; the 4 chunk-calls of each group run on 4 Q7 cpu pairs
  concurrently (queue_num=cc), with deep tile pools to keep them fed.
- Table is split lo/hi with one AllGather each, so edge gathers start
  after only the half-AG they read.
- Trailing index padding is -1: the gather ucode skips those descriptors.
- Layer-1's dense phase is interleaved into layer-0's edge loop, and the
  lo-half AllGather of layer 1 fires mid-way through layer 0.
"""
import sys
import types

import numpy as np

sys.path.insert(0, "/opt/trn_rl_repo")

N = 100000
E = 1600000
IN = 165
H = 128
NC = 8
SH = 12500
SHP = 12544           # 98 * 128
GRP = 512
NGRP = 25             # 24x512 + 1x256 slots
GW = [512] * 24 + [256]
G24_CAP = 212         # nodes assigned to group 24 (256 slots, 44+ empty)
# table split into 4 group-aligned quarters; each quarter is one gather
# chunk (rows/core: 3072,3072,3072,3328 -> totals < 32768, int16-safe)
QGRPS = [(0, 6), (6, 12), (12, 18), (18, 25)]
QROWS = [(b2 * GRP if b2 < 25 else SHP) - b1 * GRP for b1, b2 in QGRPS]
QBASE = [b1 * GRP for b1, _ in QGRPS]
QTOT = [NC * r for r in QROWS]
NCHK = 4
SSLW = 2112           # max S stream cols per call
MTB = 6               # message-tile pipeline depth
SLB = 4               # S-stream pipeline depth


def _install_ntff_hook():
    if "antenv.axon_hooks" in sys.modules:
        return
    try:
        import antenv
        mod = types.ModuleType("antenv.axon_hooks")
        mod._hook = None
        mod.set_axon_ntff_profile_hook = lambda h: setattr(mod, "_hook", h)
        mod.get_axon_ntff_profile_hook = lambda: mod._hook
        sys.modules["antenv.axon_hooks"] = mod
        antenv.axon_hooks = mod
        from trn_agent_boot.trn_boot import _ntff_profile_via_ctypes
        mod.set_axon_ntff_profile_hook(
            _ntff_profile_via_ctypes("/opt/axon/libaxon_pjrt.so"))
    except Exception:
        pass


def _split_excess_waits(nc, bass, max_waits=1, kinds=("InstDrain",)):
    """This walrus build rejects InstDrain with >1 sem waits; hoist extras
    onto standalone event-semaphore instructions placed just before."""
    wait_op_map = {"sem-ge-imm": "sem-ge", "sem-eq-imm": "sem-eq"}
    for bb in nc.main_func.blocks:
        insts = bb.instructions
        i = 0
        while i < len(insts):
            ins = insts[i]
            if (type(ins).__name__ in kinds and ins.sync_info is not None
                    and len(ins.sync_info.on_wait) > max_waits):
                waits = list(ins.sync_info.on_wait)
                ins.sync_info.on_wait = waits[:max_waits]
                eng = nc.engines[ins.engine]
                new_insts = []
                for w in waits[max_waits:]:
                    sem = bass.SemaphoreHandle(w.ant_name or "s", w.id)
                    bi = eng.wait_op(sem, w.wait_value, wait_op_map[w.wait_mode])
                    popped = None
                    for b2 in nc.main_func.blocks:
                        if b2.instructions and b2.instructions[-1] is bi.ins:
                            popped = b2.instructions.pop()
                            break
                    assert popped is bi.ins
                    new_insts.append(popped)
                for k, ni in enumerate(new_insts):
                    insts.insert(i + k, ni)
                i += len(new_insts)
            i += 1


def _balance_slots(dst_count_local):
    """Greedy bin-pack local nodes (by incoming-edge count, desc) into 25
    groups: caps 512 x24 + 212. Returns slot[node_local]."""
    nloc = dst_count_local.shape[0]
    caps = np.array([GRP] * 24 + [G24_CAP])
    order = np.argsort(-dst_count_local, kind="stable")
    loads = np.zeros(NGRP, np.int64)
    fill = np.zeros(NGRP, np.int64)
    slot = np.zeros(nloc, np.int64)
    import heapq
    heap = [(0, g) for g in range(NGRP)]
    heapq.heapify(heap)
    for v in order:
        while True:
            ld, g = heapq.heappop(heap)
            if fill[g] < caps[g]:
                break
        slot[v] = g * GRP + fill[g]
        fill[g] += 1
        loads[g] += dst_count_local[v]
        if fill[g] < caps[g]:
            heapq.heappush(heap, (loads[g], g))
    return slot


def _table_pos(slot_global_core, slot_in_core):
    """Map (core, slot) -> (chunk id 0..3, idx within chunk)."""
    c = slot_global_core
    s = slot_in_core
    cc = np.zeros_like(s)
    idx = np.zeros_like(s)
    for q in range(NCHK):
        m = (s >= QBASE[q]) & (s < QBASE[q] + QROWS[q])
        cc[m] = q
        idx[m] = c[m] * QROWS[q] + (s[m] - QBASE[q])
    return cc.astype(np.int64), idx.astype(np.int64)


def _preprocess(edge_index):
    """Build per-core slot permutations, gather index streams and selection
    matrix streams with uniform SPMD structure + -1 tail padding."""
    e0 = np.asarray(edge_index[0], dtype=np.int64)
    e1 = np.asarray(edge_index[1], dtype=np.int64)
    deg = np.bincount(e1, minlength=N).astype(np.float64) + 1.0
    dinv = (1.0 / np.sqrt(deg)).astype(np.float32)

    # per-core degree-balancing permutation node -> slot
    slot_of = np.zeros(N, np.int64)
    node_at = np.full((NC, SHP), -1, np.int64)
    for c in range(NC):
        ids = np.arange(c * SH, (c + 1) * SH)
        sl = _balance_slots(deg[ids].astype(np.int64))
        slot_of[ids] = sl
        node_at[c, sl] = ids

    src = np.concatenate([e0, np.arange(N, dtype=np.int64)])
    dst = np.concatenate([e1, np.arange(N, dtype=np.int64)])
    w_all = dinv[dst]

    src_core = src // SH
    src_slot = slot_of[src]
    src_cc, src_idx = _table_pos(src_core, src_slot)
    dst_core = dst // SH
    dst_slot = slot_of[dst]

    # per-core sorted edge streams
    streams = []
    for c in range(NC):
        m = dst_core == c
        gslot = dst_slot[m]
        g = gslot // GRP
        cc = src_cc[m]
        idx = src_idx[m]
        ww = w_all[m].astype(np.float32)
        order = np.lexsort((idx, gslot, cc, g))
        streams.append((g[order], cc[order], gslot[order], idx[order],
                        ww[order]))

    # uniform static sizes per (g, cc)
    key_list = [(g, cc) for g in range(NGRP) for cc in range(NCHK)]
    stat = {}
    for kk in key_list:
        mx = 0
        for (g, cc, gs, ix, ww) in streams:
            n = int(np.sum((g == kk[0]) & (cc == kk[1])))
            mx = max(mx, n)
        stat[kk] = ((mx + 127) // 128) * 128
    nwmax = max(v // 128 for v in stat.values())

    # window dst-col bounds unioned across cores
    win_b = {}
    percore_calls = []
    for (g, cc, gs, ix, ww) in streams:
        bycall = {}
        for kk in key_list:
            m = (g == kk[0]) & (cc == kk[1])
            bycall[kk] = (gs[m], ix[m], ww[m])
        percore_calls.append(bycall)
    for kk in key_list:
        nww = stat[kk] // 128
        for wi in range(nww):
            lo, hi = None, None
            for bycall in percore_calls:
                gs, ix, ww = bycall[kk]
                seg = gs[wi * 128:(wi + 1) * 128] - kk[0] * GRP
                if len(seg):
                    l, h = int(seg.min()), int(seg.max()) + 1
                    lo = l if lo is None else min(lo, l)
                    hi = h if hi is None else max(hi, h)
            if lo is None:
                lo, hi = 0, 1
            win_b[(kk, wi)] = (lo, hi - lo)

    # build uniform streams
    calls = []
    ioff = 0
    soff = 0
    for kk in key_list:
        ntot = stat[kk]
        nww = ntot // 128
        windows = []
        for wi in range(nww):
            o_u, Wp = win_b[(kk, wi)]
            windows.append((soff, Wp, o_u))
            soff += Wp
        calls.append((kk[0], kk[1], ioff, ntot, windows))
        ioff += ntot
    n_idx, n_scols = ioff, soff

    idx_arrs, s_arrs, ntrim_arrs = [], [], []
    for bycall in percore_calls:
        idx_cols = []
        s_cols = []
        ntrim_core = []
        for kk in key_list:
            ntot = stat[kk]
            nww = ntot // 128
            gs, ix, ww = bycall[kk]
            n = len(ix)
            # pad with index 0 (valid row, zero S weight). -1 trim desyncs
            # decode-side ring bookkeeping unless num_idxs_reg is a runtime
            # register, and per-call value_loads exhaust the register file.
            iful = np.concatenate([ix, np.zeros(ntot - n, np.int64)])
            idx_cols.append(iful.astype(np.int16))
            ntrim_core.append(n)
            for wi in range(nww):
                o_u, Wp = win_b[(kk, wi)]
                sb = np.zeros((128, Wp), np.float32)
                seg_d = gs[wi * 128:(wi + 1) * 128] - kk[0] * GRP
                seg_w = ww[wi * 128:(wi + 1) * 128]
                if len(seg_d):
                    sb[np.arange(len(seg_d)), seg_d - o_u] = seg_w
                s_cols.append(sb)
        idx_flat = np.concatenate(idx_cols)
        idx_arrs.append(np.tile(idx_flat.reshape(-1, 16).T, (NC, 1)).copy())
        s_arrs.append(np.ascontiguousarray(np.concatenate(s_cols, axis=1)))
        ntrim_arrs.append(np.array(ntrim_core, np.int32).reshape(1, -1))

    meta = dict(calls=calls, n_idx=n_idx, n_scols=n_scols, nwmax=nwmax)
    return dinv, node_at, meta, idx_arrs, s_arrs, ntrim_arrs


def _build_program(bass, bacc, mybir, tile, meta):
    nc = bacc.Bacc("TRN2", target_bir_lowering=False, debug=False,
                   num_swdge_queues=4)
    dt = mybir.dt
    f32 = dt.float32

    def din(name, shape, dtype=f32):
        return nc.dram_tensor(name, shape, dtype, kind="ExternalInput").ap()

    n_idx, n_scols, calls = meta["n_idx"], meta["n_scols"], meta["calls"]
    nwmax = meta["nwmax"]
    bycall = {(cl[0], cl[1]): cl for cl in calls}
    for cl in calls:
        scw = cl[4][-1][0] + cl[4][-1][1] - cl[4][0][0]
        assert scw <= SSLW, scw
    assert nwmax * 128 <= 4096

    xT_hi = din("xT_hi", [128, SHP])
    xT_lo = din("xT_lo", [IN - 128, SHP])
    W0T_hi = din("W0T_hi", [128, H])
    W0T_lo = din("W0T_lo", [IN - 128, H])
    W1T = din("W1T", [H, H])
    WihT = [din(f"WihT{li}", [H, 3 * H]) for li in range(2)]
    WlinT = din("WlinT", [H, 2])
    bcol = [din(f"bcol{li}", [128, 1]) for li in range(2)]
    brc = [din(f"brc{li}", [128, 1]) for li in range(2)]
    bzc = [din(f"bzc{li}", [128, 1]) for li in range(2)]
    bnc = [din(f"bnc{li}", [128, 1]) for li in range(2)]
    bhnc = [din(f"bhnc{li}", [128, 1]) for li in range(2)]
    dinv_col = din("dinv_col", [128, SHP // 128])
    blin_t = din("blin_t", [128, 2])
    idx_d = din("idx_d", [128, n_idx // 16], dt.int16)
    s_d = din("s_d", [128, n_scols])
    ntrim_d = din("ntrim_d", [1, NGRP * NCHK], dt.int32)

    out = nc.dram_tensor("out", [128, (SHP // 128) * 2], f32,
                         kind="ExternalOutput").ap()

    tsh = [[nc.dram_tensor(f"tsh{li}q{q}", [QROWS[q], H], f32)
            for q in range(NCHK)] for li in range(2)]
    Ttab = [[nc.dram_tensor(f"Ttab{li}q{q}", [QTOT[q], H], f32,
                            addr_space="Shared") for q in range(NCHK)]
            for li in range(2)]

    from concourse.masks import make_identity

    with tile.TileContext(nc) as tc:
        with (
            tc.tile_pool(name="const", bufs=1) as cp,
            tc.tile_pool(name="dsp", bufs=2) as dsp,      # dense-phase sbuf
            tc.tile_pool(name="gsp", bufs=2) as gsp,      # GRU-phase sbuf
            tc.tile_pool(name="big", bufs=1) as bigp,
            tc.tile_pool(name="mtp", bufs=MTB) as mtp,    # message tiles
            tc.tile_pool(name="slp", bufs=SLB) as slp,    # S streams
            tc.tile_pool(name="itp", bufs=10) as itp,     # idx tiles
            tc.tile_pool(name="pt", bufs=2, space="PSUM") as ptp,
            tc.tile_pool(name="pagg", bufs=3, space="PSUM") as paggp,
            tc.tile_pool(name="pg", bufs=1, space="PSUM") as pgp,
            tc.tile_pool(name="ptr", bufs=2, space="PSUM") as ptrp,
            tc.tile_pool(name="dram", bufs=1, space="DRAM") as _dp,
        ):
            ident = cp.tile([128, 128], f32)
            make_identity(nc, ident[:])
            w0hi = cp.tile([128, H], f32)
            nc.sync.dma_start(out=w0hi[:], in_=W0T_hi[:])
            w0lo = cp.tile([IN - 128, H], f32)
            nc.sync.dma_start(out=w0lo[:], in_=W0T_lo[:])
            w1 = cp.tile([H, H], f32)
            nc.sync.dma_start(out=w1[:], in_=W1T[:])
            wih = [cp.tile([H, 3 * H], f32, name=f"wih{li}") for li in range(2)]
            for li in range(2):
                nc.sync.dma_start(out=wih[li][:], in_=WihT[li][:])
            wlin = cp.tile([H, 2], f32)
            nc.sync.dma_start(out=wlin[:], in_=WlinT[:])
            bc = [cp.tile([128, 1], f32, name=f"bc{li}") for li in range(2)]
            br = [cp.tile([128, 1], f32, name=f"br{li}") for li in range(2)]
            bz = [cp.tile([128, 1], f32, name=f"bz{li}") for li in range(2)]
            bn = [cp.tile([128, 1], f32, name=f"bn{li}") for li in range(2)]
            bhn = [cp.tile([128, 1], f32, name=f"bhn{li}") for li in range(2)]
            for li in range(2):
                nc.sync.dma_start(out=bc[li][:], in_=bcol[li][:])
                nc.sync.dma_start(out=br[li][:], in_=brc[li][:])
                nc.sync.dma_start(out=bz[li][:], in_=bzc[li][:])
                nc.sync.dma_start(out=bn[li][:], in_=bnc[li][:])
                nc.sync.dma_start(out=bhn[li][:], in_=bhnc[li][:])
            dvc = cp.tile([128, SHP // 128], f32)
            nc.sync.dma_start(out=dvc[:], in_=dinv_col[:])
            blt = cp.tile([128, 2], f32)
            nc.sync.dma_start(out=blt[:], in_=blin_t[:])


            hT = bigp.tile([128, SHP], f32, tag="hT")
            y_sb = bigp.tile([128, (SHP // 128) * 2], f32, tag="ysb")

            ACT = mybir.ActivationFunctionType

            # clear message buffers once so pad-window matmuls (zero S
            # weights) never multiply uninitialized SBUF bit patterns.
            for _ in range(MTB):
                mtz = mtp.tile([128, nwmax * H], f32, tag="mt")
                nc.vector.memset(mtz[:], 0.0)

            def dense_group(li, g):
                """t.T = W @ h.T; transpose; scale dinv; DMA to table shard."""
                gw = GW[g]
                g0 = g * GRP
                pt = ptp.tile([128, GRP], f32, tag="pt")
                if li == 0:
                    xh = dsp.tile([128, GRP], f32, tag="xh")
                    nc.sync.dma_start(out=xh[:, :gw], in_=xT_hi[:, g0:g0 + gw])
                    xl = dsp.tile([IN - 128, GRP], f32, tag="xl")
                    nc.scalar.dma_start(out=xl[:, :gw],
                                        in_=xT_lo[:, g0:g0 + gw])
                    nc.tensor.matmul(pt[:, :gw], w0hi[:], xh[:, :gw],
                                     start=True, stop=False)
                    nc.tensor.matmul(pt[:, :gw], w0lo[:], xl[:, :gw],
                                     start=False, stop=True)
                else:
                    nc.tensor.matmul(pt[:, :gw], w1[:], hT[:, g0:g0 + gw],
                                     start=True, stop=True)
                tsb = dsp.tile([128, GRP], f32, tag="tsb")
                nc.scalar.copy(tsb[:, :gw], pt[:, :gw])
                for t in range(gw // 128):
                    ptr = ptrp.tile([128, 128], f32, tag="ptr")
                    nc.tensor.transpose(
                        out=ptr[:], in_=tsb[:, 128 * t:128 * (t + 1)],
                        identity=ident[:])
                    trs = dsp.tile([128, 128], f32, tag="trs")
                    col = g * (GRP // 128) + t
                    nc.vector.tensor_scalar_mul(
                        trs[:], ptr[:], dvc[:, col:col + 1])
                    r0 = g0 + 128 * t
                    q = next(i for i in range(NCHK)
                             if QBASE[i] <= r0 < QBASE[i] + QROWS[i])
                    rq = r0 - QBASE[q]
                    nc.sync.dma_start(
                        out=tsh[li][q][rq:rq + 128, :], in_=trs[:])

            def edge_calls(li, g, pagg, ccs):
                gw = GW[g]
                for cc in ccs:
                    (_, _, ioff, ntot, windows) = bycall[(g, cc)]
                    soff0 = windows[0][0]
                    scw = windows[-1][0] + windows[-1][1] - soff0
                    ssl = slp.tile([128, SSLW], f32, tag="ssl")
                    nc.scalar.dma_start(out=ssl[:, :scw],
                                        in_=s_d[:, soff0:soff0 + scw])
                    nw = ntot // 128
                    it = itp.tile([128, nwmax * 8], dt.int16, tag="it")
                    nc.sync.dma_start(
                        out=it[:, :ntot // 16],
                        in_=idx_d[:, ioff // 16:(ioff + ntot) // 16])
                    mt = mtp.tile([128, nwmax * H], f32, tag="mt")
                    nc.gpsimd.dma_gather(
                        out_ap=mt[:, :nw * H].rearrange(
                            "p (c d) -> p c d", d=H),
                        in_ap=Ttab[li][cc][:],
                        idxs_ap=it[:, :ntot // 16],
                        num_idxs=ntot, num_idxs_reg=ntot,
                        elem_size=H,
                        single_packet=False,
                        queue_num=cc)
                    for k, (soff, W, o) in enumerate(windows):
                        nc.tensor.matmul(
                            pagg[:, o:o + W],
                            mt[:, k * H:(k + 1) * H],
                            ssl[:, soff - soff0:soff - soff0 + W],
                            start=False, stop=False)

            def epilogue(li, g, pagg):
                gw = GW[g]
                g0 = g * GRP
                hg = gsp.tile([128, GRP], f32, tag="hg")
                nc.scalar.activation(hg[:, :gw], pagg[:, :gw], ACT.Relu,
                                     bias=bc[li][:], scale=1.0)
                pgr = pgp.tile([128, GRP], f32, tag="pg")
                nc.tensor.matmul(pgr[:, :gw], wih[li][:, 0:H],
                                 hg[:, :gw], start=True, stop=True)
                rt = gsp.tile([128, GRP], f32, tag="rt")
                nc.scalar.activation(rt[:, :gw], pgr[:, :gw], ACT.Sigmoid,
                                     bias=br[li][:], scale=1.0)
                pgz = pgp.tile([128, GRP], f32, tag="pg")
                nc.tensor.matmul(pgz[:, :gw], wih[li][:, H:2 * H],
                                 hg[:, :gw], start=True, stop=True)
                zt = gsp.tile([128, GRP], f32, tag="zt")
                nc.scalar.activation(zt[:, :gw], pgz[:, :gw], ACT.Sigmoid,
                                     bias=bz[li][:], scale=-1.0)
                pgn = pgp.tile([128, GRP], f32, tag="pg")
                nc.tensor.matmul(pgn[:, :gw], wih[li][:, 2 * H:3 * H],
                                 hg[:, :gw], start=True, stop=True)
                tmp = gsp.tile([128, GRP], f32, tag="st")
                nc.vector.tensor_scalar_mul(tmp[:, :gw], rt[:, :gw],
                                            bhn[li][:])
                st = gsp.tile([128, GRP], f32, tag="st")
                nc.vector.tensor_add(st[:, :gw], pgn[:, :gw], tmp[:, :gw])
                nt = gsp.tile([128, GRP], f32, tag="nt")
                nc.scalar.activation(nt[:, :gw], st[:, :gw], ACT.Tanh,
                                     bias=bn[li][:], scale=1.0)
                if li == 0:
                    nc.vector.tensor_mul(hT[:, g0:g0 + gw], zt[:, :gw],
                                         nt[:, :gw])
                else:
                    h2 = gsp.tile([128, GRP], f32, tag="h2")
                    nc.vector.tensor_mul(h2[:, :gw], zt[:, :gw], nt[:, :gw])
                    for t in range(gw // 128):
                        py = ptrp.tile([128, 128], f32, tag="ptr")
                        nc.tensor.matmul(py[:, :2],
                                         h2[:, 128 * t:128 * (t + 1)],
                                         wlin[:], start=True, stop=True)
                        col = g * (GRP // 128) + t
                        nc.vector.tensor_add(
                            y_sb[:, 2 * col:2 * col + 2], py[:, :2], blt[:])

            def allgather(li, q):
                nc.gpsimd.collective_compute(
                    "AllGather", mybir.AluOpType.bypass,
                    replica_groups=[list(range(NC))],
                    ins=[tsh[li][q][:]], outs=[Ttab[li][q][:]])

            qa_of_g = {b2 - 1: q for q, (b1, b2) in enumerate(QGRPS)}

            def new_pagg(pool):
                # ptp's buffers are reused (tag "pt") during layer-1 stagger
                # when no dense phase runs; a fresh tag would cost PSUM banks.
                tag = "pagg" if pool is paggp else "pt"
                pagg = pool.tile([128, GRP], f32, tag=tag)
                nc.vector.memset(pagg[:], 0.0)
                return pagg

            # ---- layer 0 dense; AG each quarter as its groups complete ----
            for g in range(NGRP):
                dense_group(0, g)
                if g in qa_of_g:
                    allgather(0, qa_of_g[g])

            # ---- layer 0 edge phase ----
            # Groups 12..24 first so layer 1's chunk-2/3 AGs fire mid-layer-0.
            # Head stagger: 5 groups advance chunk-by-chunk as quarter AGs
            # land (cc waves); their dense1 is deferred until the borrowed
            # pt-pool paggs are closed.
            EORD0 = list(range(12, NGRP)) + list(range(12))
            STAG0 = 5
            sgs = EORD0[:STAG0]
            paggs = {}
            for i, g in enumerate(sgs):
                paggs[g] = new_pagg(paggp if i < 3 else ptp)
            for cc in range(NCHK):
                for g in sgs:
                    edge_calls(0, g, paggs[g], [cc])
            for g in sgs:
                epilogue(0, g, paggs[g])
                del paggs[g]
            for g in sgs:
                dense_group(1, g)
            for g in EORD0[STAG0:]:
                pagg = new_pagg(paggp)
                edge_calls(0, g, pagg, [0, 1, 2, 3])
                epilogue(0, g, pagg)
                dense_group(1, g)
                if g in (17, 24, 5, 11):
                    allgather(1, {17: 2, 24: 3, 5: 0, 11: 1}[g])

            # ---- layer 1 edge phase: stagger 5 groups, chunk-readiness
            # order [2,3] (AGs done mid-layer-0), then 0, then 1 (its AG
            # fires at layer-0 end). 2 paggs borrow the pt pool. ----
            STAG1 = 5
            paggs = {}
            for g in range(STAG1):
                paggs[g] = new_pagg(paggp if g < 3 else ptp)
                edge_calls(1, g, paggs[g], [2, 3])
            for g in range(STAG1):
                edge_calls(1, g, paggs[g], [0])
            for g in range(STAG1):
                edge_calls(1, g, paggs[g], [1])
                epilogue(1, g, paggs[g])
                del paggs[g]
            for g in range(STAG1, NGRP):
                pagg = new_pagg(paggp)
                edge_calls(1, g, pagg, [0, 1, 2, 3])
                epilogue(1, g, pagg)
            nc.sync.dma_start(out=out[:], in_=y_sb[:])

    _split_excess_waits(nc, bass)
    nc.finalize()
    return nc


def kernel(**inputs):
    _install_ntff_hook()
    import concourse.bass as bass
    import concourse.bacc as bacc
    import concourse.mybir as mybir
    import concourse.tile as tile
    from concourse.bass_utils import run_bass_kernel_spmd

    x = np.asarray(inputs["x"], np.float32)
    edge_index = np.asarray(inputs["edge_index"])
    (dinv, node_at, meta, idx_arrs, s_arrs,
     ntrim_arrs) = _preprocess(edge_index)

    nc = _build_program(bass, bacc, mybir, tile, meta)

    W0 = np.asarray(inputs["W0"], np.float32)
    W1 = np.asarray(inputs["W1"], np.float32)
    Wlin = np.asarray(inputs["Wlin"], np.float32)
    in_maps = []
    for c in range(NC):
        na = node_at[c]
        filled = na >= 0
        xs = np.zeros((SHP, IN), np.float32)
        xs[filled] = x[na[filled]]
        xT = np.ascontiguousarray(xs.T)
        bias_stage = {}
        for li in range(2):
            bih = np.asarray(inputs[f"bih{li}"], np.float32)
            bhh = np.asarray(inputs[f"bhh{li}"], np.float32)
            bias_stage[f"bcol{li}"] = np.asarray(
                inputs[f"b{li}"], np.float32).reshape(128, 1)
            bias_stage[f"brc{li}"] = (bih[:H] + bhh[:H]).reshape(128, 1)
            bias_stage[f"bzc{li}"] = (
                -(bih[H:2 * H] + bhh[H:2 * H])).reshape(128, 1)
            bias_stage[f"bnc{li}"] = bih[2 * H:].reshape(128, 1)
            bias_stage[f"bhnc{li}"] = bhh[2 * H:].reshape(128, 1)
        dv = np.zeros(SHP, np.float32)
        dv[filled] = dinv[na[filled]]
        in_maps.append({
            "xT_hi": xT[:128], "xT_lo": xT[128:],
            "W0T_hi": np.ascontiguousarray(W0.T[:128]),
            "W0T_lo": np.ascontiguousarray(W0.T[128:]),
            "W1T": np.ascontiguousarray(W1.T),
            "WihT0": np.ascontiguousarray(
                np.asarray(inputs["Wih0"], np.float32).T),
            "WihT1": np.ascontiguousarray(
                np.asarray(inputs["Wih1"], np.float32).T),
            "WlinT": np.ascontiguousarray(Wlin.T),
            **bias_stage,
            "dinv_col": np.ascontiguousarray(
                dv.reshape(SHP // 128, 128).T),
            "blin_t": np.tile(np.asarray(inputs["blin"], np.float32),
                              (128, 1)),
            "idx_d": idx_arrs[c],
            "s_d": s_arrs[c],
            "ntrim_d": ntrim_arrs[c],
        })

    res = run_bass_kernel_spmd(nc, in_maps, list(range(NC)),
                               trace=bool(int(__import__("os").environ.get(
                                   "KERNEL_TRACE", "0"))))
    kernel.last_results = res
    y = np.zeros((N, 2), np.float32)
    for c in range(NC):
        o = res.results[c]["out"]  # [128, 98*2]
        yy = o.reshape(128, SHP // 128, 2).transpose(1, 0, 2).reshape(SHP, 2)
        na = node_at[c]
        filled = na >= 0
        y[na[filled]] = yy[filled]
    return y


# revision 40
# speedup vs baseline: 1.1198x; 1.1198x over previous
"""EvolveGCN (2x GCNConv+GRU + linear head) on 8 Trainium2 NeuronCores.

Strategy: dst-sharded graph parallel, gather-based message routing.
Each core owns 12500 destination nodes mapped onto 12544 slots by a
degree-balancing permutation (equalizes edge count per (group, chunk)
bucket across cores, minimizing SPMD padding). Dense per-node compute is
feature-major. The GCN propagate runs as PE matmuls over 128-edge
windows: psum[128f, W] += M_w.T @ S_w where M_w is a dma_gather'ed
message tile and S_w a host-built selection matrix.

Key perf structure (the kernel is Q7 descriptor-generation bound):
- 4 SWDGE queues; the 4 chunk-calls of each group run on 4 Q7 cpu pairs
  concurrently (queue_num=cc), with deep tile pools to keep them fed.
- Table is split lo/hi with one AllGather each, so edge gathers start
  after only the half-AG they read.
- Trailing index padding is -1: the gather ucode skips those descriptors.
- Layer-1's dense phase is interleaved into layer-0's edge loop, and the
  lo-half AllGather of layer 1 fires mid-way through layer 0.
"""
import sys
import types

import numpy as np

sys.path.insert(0, "/opt/trn_rl_repo")

N = 100000
E = 1600000
IN = 165
H = 128
NC = 8
SH = 12500
SHP = 12544           # 98 * 128
GRP = 512
NGRP = 25             # 24x512 + 1x256 slots
GW = [512] * 24 + [256]
G24_CAP = 212         # nodes assigned to group 24 (256 slots, 44+ empty)
# table split into 4 group-aligned quarters; each quarter is one gather
# chunk (rows/core: 3072,3072,3072,3328 -> totals < 32768, int16-safe)
QGRPS = [(0, 6), (6, 12), (12, 18), (18, 25)]
QROWS = [(b2 * GRP if b2 < 25 else SHP) - b1 * GRP for b1, b2 in QGRPS]
QBASE = [b1 * GRP for b1, _ in QGRPS]
QTOT = [NC * r for r in QROWS]
NCHK = 4
SSLW = 2112           # max S stream cols per call
MTB = 6               # message-tile pipeline depth
SLB = 4               # S-stream pipeline depth


def _install_ntff_hook():
    if "antenv.axon_hooks" in sys.modules:
        return
    try:
        import antenv
        mod = types.ModuleType("antenv.axon_hooks")
        mod._hook = None
        mod.set_axon_ntff_profile_hook = lambda h: setattr(mod, "_hook", h)
        mod.get_axon_ntff_profile_hook = lambda: mod._hook
        sys.modules["antenv.axon_hooks"] = mod
        antenv.axon_hooks = mod
        from trn_agent_boot.trn_boot import _ntff_profile_via_ctypes
        mod.set_axon_ntff_profile_hook(
            _ntff_profile_via_ctypes("/opt/axon/libaxon_pjrt.so"))
    except Exception:
        pass


def _split_excess_waits(nc, bass, max_waits=1, kinds=("InstDrain",)):
    """This walrus build rejects InstDrain with >1 sem waits; hoist extras
    onto standalone event-semaphore instructions placed just before."""
    wait_op_map = {"sem-ge-imm": "sem-ge", "sem-eq-imm": "sem-eq"}
    for bb in nc.main_func.blocks:
        insts = bb.instructions
        i = 0
        while i < len(insts):
            ins = insts[i]
            if (type(ins).__name__ in kinds and ins.sync_info is not None
                    and len(ins.sync_info.on_wait) > max_waits):
                waits = list(ins.sync_info.on_wait)
                ins.sync_info.on_wait = waits[:max_waits]
                eng = nc.engines[ins.engine]
                new_insts = []
                for w in waits[max_waits:]:
                    sem = bass.SemaphoreHandle(w.ant_name or "s", w.id)
                    bi = eng.wait_op(sem, w.wait_value, wait_op_map[w.wait_mode])
                    popped = None
                    for b2 in nc.main_func.blocks:
                        if b2.instructions and b2.instructions[-1] is bi.ins:
                            popped = b2.instructions.pop()
                            break
                    assert popped is bi.ins
                    new_insts.append(popped)
                for k, ni in enumerate(new_insts):
                    insts.insert(i + k, ni)
                i += len(new_insts)
            i += 1


def _balance_slots(dst_count_local):
    """Greedy bin-pack local nodes (by incoming-edge count, desc) into 25
    groups: caps 512 x24 + 212. Returns slot[node_local]."""
    nloc = dst_count_local.shape[0]
    caps = np.array([GRP] * 24 + [G24_CAP])
    order = np.argsort(-dst_count_local, kind="stable")
    loads = np.zeros(NGRP, np.int64)
    fill = np.zeros(NGRP, np.int64)
    slot = np.zeros(nloc, np.int64)
    import heapq
    heap = [(0, g) for g in range(NGRP)]
    heapq.heapify(heap)
    for v in order:
        while True:
            ld, g = heapq.heappop(heap)
            if fill[g] < caps[g]:
                break
        slot[v] = g * GRP + fill[g]
        fill[g] += 1
        loads[g] += dst_count_local[v]
        if fill[g] < caps[g]:
            heapq.heappush(heap, (loads[g], g))
    return slot


def _table_pos(slot_global_core, slot_in_core):
    """Map (core, slot) -> (chunk id 0..3, idx within chunk)."""
    c = slot_global_core
    s = slot_in_core
    cc = np.zeros_like(s)
    idx = np.zeros_like(s)
    for q in range(NCHK):
        m = (s >= QBASE[q]) & (s < QBASE[q] + QROWS[q])
        cc[m] = q
        idx[m] = c[m] * QROWS[q] + (s[m] - QBASE[q])
    return cc.astype(np.int64), idx.astype(np.int64)


def _preprocess(edge_index):
    """Build per-core slot permutations, gather index streams and selection
    matrix streams with uniform SPMD structure + -1 tail padding."""
    e0 = np.asarray(edge_index[0], dtype=np.int64)
    e1 = np.asarray(edge_index[1], dtype=np.int64)
    deg = np.bincount(e1, minlength=N).astype(np.float64) + 1.0
    dinv = (1.0 / np.sqrt(deg)).astype(np.float32)

    # per-core degree-balancing permutation node -> slot
    slot_of = np.zeros(N, np.int64)
    node_at = np.full((NC, SHP), -1, np.int64)
    for c in range(NC):
        ids = np.arange(c * SH, (c + 1) * SH)
        sl = _balance_slots(deg[ids].astype(np.int64))
        slot_of[ids] = sl
        node_at[c, sl] = ids

    src = np.concatenate([e0, np.arange(N, dtype=np.int64)])
    dst = np.concatenate([e1, np.arange(N, dtype=np.int64)])
    w_all = dinv[dst]

    src_core = src // SH
    src_slot = slot_of[src]
    src_cc, src_idx = _table_pos(src_core, src_slot)
    dst_core = dst // SH
    dst_slot = slot_of[dst]

    # per-core sorted edge streams
    streams = []
    for c in range(NC):
        m = dst_core == c
        gslot = dst_slot[m]
        g = gslot // GRP
        cc = src_cc[m]
        idx = src_idx[m]
        ww = w_all[m].astype(np.float32)
        order = np.lexsort((idx, gslot, cc, g))
        streams.append((g[order], cc[order], gslot[order], idx[order],
                        ww[order]))

    # uniform static sizes per (g, cc)
    key_list = [(g, cc) for g in range(NGRP) for cc in range(NCHK)]
    stat = {}
    for kk in key_list:
        mx = 0
        for (g, cc, gs, ix, ww) in streams:
            n = int(np.sum((g == kk[0]) & (cc == kk[1])))
            mx = max(mx, n)
        stat[kk] = ((mx + 127) // 128) * 128
    nwmax = max(v // 128 for v in stat.values())

    # window dst-col bounds unioned across cores
    win_b = {}
    percore_calls = []
    for (g, cc, gs, ix, ww) in streams:
        bycall = {}
        for kk in key_list:
            m = (g == kk[0]) & (cc == kk[1])
            bycall[kk] = (gs[m], ix[m], ww[m])
        percore_calls.append(bycall)
    for kk in key_list:
        nww = stat[kk] // 128
        for wi in range(nww):
            lo, hi = None, None
            for bycall in percore_calls:
                gs, ix, ww = bycall[kk]
                seg = gs[wi * 128:(wi + 1) * 128] - kk[0] * GRP
                if len(seg):
                    l, h = int(seg.min()), int(seg.max()) + 1
                    lo = l if lo is None else min(lo, l)
                    hi = h if hi is None else max(hi, h)
            if lo is None:
                lo, hi = 0, 1
            win_b[(kk, wi)] = (lo, hi - lo)

    # build uniform streams
    calls = []
    ioff = 0
    soff = 0
    for kk in key_list:
        ntot = stat[kk]
        nww = ntot // 128
        windows = []
        for wi in range(nww):
            o_u, Wp = win_b[(kk, wi)]
            windows.append((soff, Wp, o_u))
            soff += Wp
        calls.append((kk[0], kk[1], ioff, ntot, windows))
        ioff += ntot
    n_idx, n_scols = ioff, soff

    idx_arrs, s_arrs, ntrim_arrs = [], [], []
    for bycall in percore_calls:
        idx_cols = []
        s_cols = []
        ntrim_core = []
        for kk in key_list:
            ntot = stat[kk]
            nww = ntot // 128
            gs, ix, ww = bycall[kk]
            n = len(ix)
            # pad with index 0 (valid row, zero S weight). -1 trim desyncs
            # decode-side ring bookkeeping unless num_idxs_reg is a runtime
            # register, and per-call value_loads exhaust the register file.
            iful = np.concatenate([ix, np.zeros(ntot - n, np.int64)])
            idx_cols.append(iful.astype(np.int16))
            ntrim_core.append(n)
            for wi in range(nww):
                o_u, Wp = win_b[(kk, wi)]
                sb = np.zeros((128, Wp), np.float32)
                seg_d = gs[wi * 128:(wi + 1) * 128] - kk[0] * GRP
                seg_w = ww[wi * 128:(wi + 1) * 128]
                if len(seg_d):
                    sb[np.arange(len(seg_d)), seg_d - o_u] = seg_w
                s_cols.append(sb)
        idx_flat = np.concatenate(idx_cols)
        idx_arrs.append(np.tile(idx_flat.reshape(-1, 16).T, (NC, 1)).copy())
        s_arrs.append(np.ascontiguousarray(np.concatenate(s_cols, axis=1)))
        ntrim_arrs.append(np.array(ntrim_core, np.int32).reshape(1, -1))

    meta = dict(calls=calls, n_idx=n_idx, n_scols=n_scols, nwmax=nwmax)
    return dinv, node_at, meta, idx_arrs, s_arrs, ntrim_arrs


def _build_program(bass, bacc, mybir, tile, meta):
    nc = bacc.Bacc("TRN2", target_bir_lowering=False, debug=False,
                   num_swdge_queues=4)
    dt = mybir.dt
    f32 = dt.float32

    def din(name, shape, dtype=f32):
        return nc.dram_tensor(name, shape, dtype, kind="ExternalInput").ap()

    n_idx, n_scols, calls = meta["n_idx"], meta["n_scols"], meta["calls"]
    nwmax = meta["nwmax"]
    bycall = {(cl[0], cl[1]): cl for cl in calls}
    for cl in calls:
        scw = cl[4][-1][0] + cl[4][-1][1] - cl[4][0][0]
        assert scw <= SSLW, scw
    assert nwmax * 128 <= 4096

    xT_hi = din("xT_hi", [128, SHP])
    xT_lo = din("xT_lo", [IN - 128, SHP])
    W0T_hi = din("W0T_hi", [128, H])
    W0T_lo = din("W0T_lo", [IN - 128, H])
    W1T = din("W1T", [H, H])
    WihT = [din(f"WihT{li}", [H, 3 * H]) for li in range(2)]
    WlinT = din("WlinT", [H, 2])
    bcol = [din(f"bcol{li}", [128, 1]) for li in range(2)]
    brc = [din(f"brc{li}", [128, 1]) for li in range(2)]
    bzc = [din(f"bzc{li}", [128, 1]) for li in range(2)]
    bnc = [din(f"bnc{li}", [128, 1]) for li in range(2)]
    bhnc = [din(f"bhnc{li}", [128, 1]) for li in range(2)]
    dinv_col = din("dinv_col", [128, SHP // 128])
    blin_t = din("blin_t", [128, 2])
    idx_d = din("idx_d", [128, n_idx // 16], dt.int16)
    s_d = din("s_d", [128, n_scols])
    ntrim_d = din("ntrim_d", [1, NGRP * NCHK], dt.int32)

    out = nc.dram_tensor("out", [128, (SHP // 128) * 2], f32,
                         kind="ExternalOutput").ap()

    tsh = [[nc.dram_tensor(f"tsh{li}q{q}", [QROWS[q], H], f32)
            for q in range(NCHK)] for li in range(2)]
    Ttab = [[nc.dram_tensor(f"Ttab{li}q{q}", [QTOT[q], H], f32,
                            addr_space="Shared") for q in range(NCHK)]
            for li in range(2)]

    from concourse.masks import make_identity

    with tile.TileContext(nc) as tc:
        with (
            tc.tile_pool(name="const", bufs=1) as cp,
            tc.tile_pool(name="dsp", bufs=2) as dsp,      # dense-phase sbuf
            tc.tile_pool(name="gsp", bufs=2) as gsp,      # GRU-phase sbuf
            tc.tile_pool(name="big", bufs=1) as bigp,
            tc.tile_pool(name="mtp", bufs=MTB) as mtp,    # message tiles
            tc.tile_pool(name="slp", bufs=SLB) as slp,    # S streams
            tc.tile_pool(name="itp", bufs=10) as itp,     # idx tiles
            tc.tile_pool(name="pt", bufs=2, space="PSUM") as ptp,
            tc.tile_pool(name="pagg", bufs=3, space="PSUM") as paggp,
            tc.tile_pool(name="pg", bufs=1, space="PSUM") as pgp,
            tc.tile_pool(name="ptr", bufs=2, space="PSUM") as ptrp,
            tc.tile_pool(name="dram", bufs=1, space="DRAM") as _dp,
        ):
            ident = cp.tile([128, 128], f32)
            make_identity(nc, ident[:])
            w0hi = cp.tile([128, H], f32)
            nc.sync.dma_start(out=w0hi[:], in_=W0T_hi[:])
            w0lo = cp.tile([IN - 128, H], f32)
            nc.sync.dma_start(out=w0lo[:], in_=W0T_lo[:])
            w1 = cp.tile([H, H], f32)
            nc.sync.dma_start(out=w1[:], in_=W1T[:])
            wih = [cp.tile([H, 3 * H], f32, name=f"wih{li}") for li in range(2)]
            for li in range(2):
                nc.sync.dma_start(out=wih[li][:], in_=WihT[li][:])
            wlin = cp.tile([H, 2], f32)
            nc.sync.dma_start(out=wlin[:], in_=WlinT[:])
            bc = [cp.tile([128, 1], f32, name=f"bc{li}") for li in range(2)]
            br = [cp.tile([128, 1], f32, name=f"br{li}") for li in range(2)]
            bz = [cp.tile([128, 1], f32, name=f"bz{li}") for li in range(2)]
            bn = [cp.tile([128, 1], f32, name=f"bn{li}") for li in range(2)]
            bhn = [cp.tile([128, 1], f32, name=f"bhn{li}") for li in range(2)]
            for li in range(2):
                nc.sync.dma_start(out=bc[li][:], in_=bcol[li][:])
                nc.sync.dma_start(out=br[li][:], in_=brc[li][:])
                nc.sync.dma_start(out=bz[li][:], in_=bzc[li][:])
                nc.sync.dma_start(out=bn[li][:], in_=bnc[li][:])
                nc.sync.dma_start(out=bhn[li][:], in_=bhnc[li][:])
            dvc = cp.tile([128, SHP // 128], f32)
            nc.sync.dma_start(out=dvc[:], in_=dinv_col[:])
            blt = cp.tile([128, 2], f32)
            nc.sync.dma_start(out=blt[:], in_=blin_t[:])


            hT = bigp.tile([128, SHP], f32, tag="hT")
            y_sb = bigp.tile([128, (SHP // 128) * 2], f32, tag="ysb")

            ACT = mybir.ActivationFunctionType

            # clear message buffers once so pad-window matmuls (zero S
            # weights) never multiply uninitialized SBUF bit patterns.
            for _ in range(MTB):
                mtz = mtp.tile([128, nwmax * H], f32, tag="mt")
                nc.vector.memset(mtz[:], 0.0)

            def dense_group(li, g):
                """t.T = W @ h.T; transpose; scale dinv; DMA to table shard."""
                gw = GW[g]
                g0 = g * GRP
                pt = ptp.tile([128, GRP], f32, tag="pt")
                if li == 0:
                    xh = dsp.tile([128, GRP], f32, tag="xh")
                    nc.sync.dma_start(out=xh[:, :gw], in_=xT_hi[:, g0:g0 + gw])
                    xl = dsp.tile([IN - 128, GRP], f32, tag="xl")
                    nc.scalar.dma_start(out=xl[:, :gw],
                                        in_=xT_lo[:, g0:g0 + gw])
                    nc.tensor.matmul(pt[:, :gw], w0hi[:], xh[:, :gw],
                                     start=True, stop=False)
                    nc.tensor.matmul(pt[:, :gw], w0lo[:], xl[:, :gw],
                                     start=False, stop=True)
                else:
                    nc.tensor.matmul(pt[:, :gw], w1[:], hT[:, g0:g0 + gw],
                                     start=True, stop=True)
                tsb = dsp.tile([128, GRP], f32, tag="tsb")
                nc.scalar.copy(tsb[:, :gw], pt[:, :gw])
                for t in range(gw // 128):
                    ptr = ptrp.tile([128, 128], f32, tag="ptr")
                    nc.tensor.transpose(
                        out=ptr[:], in_=tsb[:, 128 * t:128 * (t + 1)],
                        identity=ident[:])
                    trs = dsp.tile([128, 128], f32, tag="trs")
                    col = g * (GRP // 128) + t
                    nc.vector.tensor_scalar_mul(
                        trs[:], ptr[:], dvc[:, col:col + 1])
                    r0 = g0 + 128 * t
                    q = next(i for i in range(NCHK)
                             if QBASE[i] <= r0 < QBASE[i] + QROWS[i])
                    rq = r0 - QBASE[q]
                    nc.sync.dma_start(
                        out=tsh[li][q][rq:rq + 128, :], in_=trs[:])

            def edge_calls(li, g, pagg, ccs):
                gw = GW[g]
                for cc in ccs:
                    (_, _, ioff, ntot, windows) = bycall[(g, cc)]
                    soff0 = windows[0][0]
                    scw = windows[-1][0] + windows[-1][1] - soff0
                    ssl = slp.tile([128, SSLW], f32, tag="ssl")
                    nc.scalar.dma_start(out=ssl[:, :scw],
                                        in_=s_d[:, soff0:soff0 + scw])
                    nw = ntot // 128
                    it = itp.tile([128, nwmax * 8], dt.int16, tag="it")
                    nc.sync.dma_start(
                        out=it[:, :ntot // 16],
                        in_=idx_d[:, ioff // 16:(ioff + ntot) // 16])
                    mt = mtp.tile([128, nwmax * H], f32, tag="mt")
                    nc.gpsimd.dma_gather(
                        out_ap=mt[:, :nw * H].rearrange(
                            "p (c d) -> p c d", d=H),
                        in_ap=Ttab[li][cc][:],
                        idxs_ap=it[:, :ntot // 16],
                        num_idxs=ntot, num_idxs_reg=ntot,
                        elem_size=H,
                        single_packet=False,
                        queue_num=cc)
                    for k, (soff, W, o) in enumerate(windows):
                        nc.tensor.matmul(
                            pagg[:, o:o + W],
                            mt[:, k * H:(k + 1) * H],
                            ssl[:, soff - soff0:soff - soff0 + W],
                            start=False, stop=False)

            def epilogue(li, g, pagg):
                gw = GW[g]
                g0 = g * GRP
                hg = gsp.tile([128, GRP], f32, tag="hg")
                nc.scalar.activation(hg[:, :gw], pagg[:, :gw], ACT.Relu,
                                     bias=bc[li][:], scale=1.0)
                pgr = pgp.tile([128, GRP], f32, tag="pg")
                nc.tensor.matmul(pgr[:, :gw], wih[li][:, 0:H],
                                 hg[:, :gw], start=True, stop=True)
                rt = gsp.tile([128, GRP], f32, tag="rt")
                nc.scalar.activation(rt[:, :gw], pgr[:, :gw], ACT.Sigmoid,
                                     bias=br[li][:], scale=1.0)
                pgz = pgp.tile([128, GRP], f32, tag="pg")
                nc.tensor.matmul(pgz[:, :gw], wih[li][:, H:2 * H],
                                 hg[:, :gw], start=True, stop=True)
                zt = gsp.tile([128, GRP], f32, tag="zt")
                nc.scalar.activation(zt[:, :gw], pgz[:, :gw], ACT.Sigmoid,
                                     bias=bz[li][:], scale=-1.0)
                pgn = pgp.tile([128, GRP], f32, tag="pg")
                nc.tensor.matmul(pgn[:, :gw], wih[li][:, 2 * H:3 * H],
                                 hg[:, :gw], start=True, stop=True)
                tmp = gsp.tile([128, GRP], f32, tag="st")
                nc.vector.tensor_scalar_mul(tmp[:, :gw], rt[:, :gw],
                                            bhn[li][:])
                st = gsp.tile([128, GRP], f32, tag="st")
                nc.vector.tensor_add(st[:, :gw], pgn[:, :gw], tmp[:, :gw])
                nt = gsp.tile([128, GRP], f32, tag="nt")
                nc.scalar.activation(nt[:, :gw], st[:, :gw], ACT.Tanh,
                                     bias=bn[li][:], scale=1.0)
                if li == 0:
                    nc.vector.tensor_mul(hT[:, g0:g0 + gw], zt[:, :gw],
                                         nt[:, :gw])
                else:
                    h2 = gsp.tile([128, GRP], f32, tag="h2")
                    nc.vector.tensor_mul(h2[:, :gw], zt[:, :gw], nt[:, :gw])
                    for t in range(gw // 128):
                        py = ptrp.tile([128, 128], f32, tag="ptr")
                        nc.tensor.matmul(py[:, :2],
                                         h2[:, 128 * t:128 * (t + 1)],
                                         wlin[:], start=True, stop=True)
                        col = g * (GRP // 128) + t
                        nc.vector.tensor_add(
                            y_sb[:, 2 * col:2 * col + 2], py[:, :2], blt[:])

            def allgather(li, q):
                nc.gpsimd.collective_compute(
                    "AllGather", mybir.AluOpType.bypass,
                    replica_groups=[list(range(NC))],
                    ins=[tsh[li][q][:]], outs=[Ttab[li][q][:]])

            qa_of_g = {b2 - 1: q for q, (b1, b2) in enumerate(QGRPS)}

            def new_pagg(pool):
                # ptp's buffers are reused (tag "pt") during layer-1 stagger
                # when no dense phase runs; a fresh tag would cost PSUM banks.
                tag = "pagg" if pool is paggp else "pt"
                pagg = pool.tile([128, GRP], f32, tag=tag)
                nc.vector.memset(pagg[:], 0.0)
                return pagg

            # ---- layer 0 dense; AG each quarter as its groups complete ----
            for g in range(NGRP):
                dense_group(0, g)
                if g in qa_of_g:
                    allgather(0, qa_of_g[g])

            # ---- layer 0 edge phase (stagger; dense1 interleaved) ----
            STAG0 = 3
            paggs = {}
            for g in range(STAG0):
                paggs[g] = new_pagg(paggp)
                edge_calls(0, g, paggs[g], [0, 1])
            for g in range(STAG0):
                edge_calls(0, g, paggs[g], [2, 3])
                epilogue(0, g, paggs[g])
                dense_group(1, g)
                del paggs[g]
            for g in range(STAG0, NGRP):
                pagg = new_pagg(paggp)
                edge_calls(0, g, pagg, [0, 1, 2, 3])
                epilogue(0, g, pagg)
                dense_group(1, g)
                if g in qa_of_g:
                    allgather(1, qa_of_g[g])

            # ---- layer 1 edge phase: stagger 5 groups; cc3 (whose AG fires
            # at layer-0 end) is deferred to the last wave. 2 paggs borrow
            # the pt pool. ----
            STAG1 = 5
            paggs = {}
            for g in range(STAG1):
                paggs[g] = new_pagg(paggp if g < 3 else ptp)
                edge_calls(1, g, paggs[g], [0, 1])
            for g in range(STAG1):
                edge_calls(1, g, paggs[g], [2])
            for g in range(STAG1):
                edge_calls(1, g, paggs[g], [3])
                epilogue(1, g, paggs[g])
                del paggs[g]
            for g in range(STAG1, NGRP):
                pagg = new_pagg(paggp)
                edge_calls(1, g, pagg, [0, 1, 2, 3])
                epilogue(1, g, pagg)
            nc.sync.dma_start(out=out[:], in_=y_sb[:])

    _split_excess_waits(nc, bass)
    nc.finalize()
    return nc


def kernel(**inputs):
    _install_ntff_hook()
    import concourse.bass as bass
    import concourse.bacc as bacc
    import concourse.mybir as mybir
    import concourse.tile as tile
    from concourse.bass_utils import run_bass_kernel_spmd

    x = np.asarray(inputs["x"], np.float32)
    edge_index = np.asarray(inputs["edge_index"])
    (dinv, node_at, meta, idx_arrs, s_arrs,
     ntrim_arrs) = _preprocess(edge_index)

    nc = _build_program(bass, bacc, mybir, tile, meta)

    W0 = np.asarray(inputs["W0"], np.float32)
    W1 = np.asarray(inputs["W1"], np.float32)
    Wlin = np.asarray(inputs["Wlin"], np.float32)
    in_maps = []
    for c in range(NC):
        na = node_at[c]
        filled = na >= 0
        xs = np.zeros((SHP, IN), np.float32)
        xs[filled] = x[na[filled]]
        xT = np.ascontiguousarray(xs.T)
        bias_stage = {}
        for li in range(2):
            bih = np.asarray(inputs[f"bih{li}"], np.float32)
            bhh = np.asarray(inputs[f"bhh{li}"], np.float32)
            bias_stage[f"bcol{li}"] = np.asarray(
                inputs[f"b{li}"], np.float32).reshape(128, 1)
            bias_stage[f"brc{li}"] = (bih[:H] + bhh[:H]).reshape(128, 1)
            bias_stage[f"bzc{li}"] = (
                -(bih[H:2 * H] + bhh[H:2 * H])).reshape(128, 1)
            bias_stage[f"bnc{li}"] = bih[2 * H:].reshape(128, 1)
            bias_stage[f"bhnc{li}"] = bhh[2 * H:].reshape(128, 1)
        dv = np.zeros(SHP, np.float32)
        dv[filled] = dinv[na[filled]]
        in_maps.append({
            "xT_hi": xT[:128], "xT_lo": xT[128:],
            "W0T_hi": np.ascontiguousarray(W0.T[:128]),
            "W0T_lo": np.ascontiguousarray(W0.T[128:]),
            "W1T": np.ascontiguousarray(W1.T),
            "WihT0": np.ascontiguousarray(
                np.asarray(inputs["Wih0"], np.float32).T),
            "WihT1": np.ascontiguousarray(
                np.asarray(inputs["Wih1"], np.float32).T),
            "WlinT": np.ascontiguousarray(Wlin.T),
            **bias_stage,
            "dinv_col": np.ascontiguousarray(
                dv.reshape(SHP // 128, 128).T),
            "blin_t": np.tile(np.asarray(inputs["blin"], np.float32),
                              (128, 1)),
            "idx_d": idx_arrs[c],
            "s_d": s_arrs[c],
            "ntrim_d": ntrim_arrs[c],
        })

    res = run_bass_kernel_spmd(nc, in_maps, list(range(NC)),
                               trace=bool(int(__import__("os").environ.get(
                                   "KERNEL_TRACE", "0"))))
    kernel.last_results = res
    y = np.zeros((N, 2), np.float32)
    for c in range(NC):
        o = res.results[c]["out"]  # [128, 98*2]
        yy = o.reshape(128, SHP // 128, 2).transpose(1, 0, 2).reshape(SHP, 2)
        na = node_at[c]
        filled = na >= 0
        y[na[filled]] = yy[filled]
    return y
